# revision 1
# baseline (speedup 1.0000x reference)
"""GWPooling2D forward on 8 Trainium2 NeuronCores.

y[b, c, o0, o1] = sum_k m[c, o0, o1, k] * x[b, k]   (k = 20*20 input pixels)

The pooling map m depends only on the small `signal` parameter
(C=16, 2, 24, 24); it is computed on host (FFTs + 16 complex 576x576
matrix exponentials) exactly as in the reference, replicated to every
core. The heavy einsum (8192 x 400 x 4096) is data-parallel across the
8 cores: each core gets a 1024-batch shard of x (pre-transposed so the
contraction dim lies on SBUF partitions) and computes its (1024, 4096)
output slab with float32r matmuls accumulating in PSUM.
"""

import numpy as np
import scipy.linalg

import concourse.bass as bass
import concourse.bacc as bacc
import concourse.mybir as mybir
import concourse.tile as tile
from concourse.bass_utils import run_bass_kernel_spmd

C = 16
P = (24, 24)
NI = (20, 20)
NO = (16, 16)
B = 8192
NCORES = 8
BS = B // NCORES              # 1024 batch rows per core
K = NI[0] * NI[1]             # 400 contraction
O = C * NO[0] * NO[1]         # 4096 output features
KP = 100                      # contraction rows per chunk (on SBUF partitions)
KC = 4                        # chunks: KP * KC == K
BT = 128                      # batch tile (PSUM partitions)
OT = 512                      # output-feature tile (PSUM free dim)


# ---------------------------------------------------------------- host map ---

def _hann(n):
    return 0.5 * (1.0 - np.cos(2.0 * np.pi * np.arange(n) / n))


def _signal_to_spectrum(signal):
    n0, n1 = signal.shape[-2], signal.shape[-1]
    window = _hann(n0)[:, None] * _hann(n1)[None, :]
    rx = np.arange((-n0) // 2 + 1, n0 // 2 + 1)[:, None]
    ry = np.arange((-n1) // 2 + 1, n1 // 2 + 1)[None, :]
    r = (1 + rx * rx + ry * ry).astype(np.float64)
    wf = np.roll(np.fft.fft2(signal), (n0 // 2, n1 // 2), (-2, -1)) / r / 5.0
    wt = np.fft.ifft2(np.roll(wf, (-(n0 // 2), -(n1 // 2)), (-2, -1))) * window
    return np.roll(np.fft.fft2(wt), (n0 // 2, n1 // 2), (-2, -1))


def _gw2d_algebra(w):
    p0, p1 = w.shape[-2], w.shape[-1]
    pad = [(0, 0)] * (w.ndim - 2) + [(p1 // 2, p1 // 2), (p0 // 2, p0 // 2)]
    wp = np.pad(w, pad)
    ia = np.arange(p0)[:, None] + np.arange(p0)[None, :]
    jb = np.arange(p1)[:, None] + np.arange(p1)[None, :]
    ws = wp[..., ia[:, None, :, None], jb[None, :, None, :]]
    ws = ws[..., ::-1, ::-1, :, :]
    kx = np.arange((-p0) // 2 + 1, p0 // 2 + 1)[:, None]
    ky = np.arange((-p1) // 2 + 1, p1 // 2 + 1)[None, :]
    return -1j * (ws[..., 0, :, :, :, :] * kx + ws[..., 1, :, :, :, :] * ky)


def _transform_to_map(t):
    p0, p1 = t.shape[-2], t.shape[-1]
    di = (p0 - NI[0], p1 - NI[1])
    do = (p0 - NO[0], p1 - NO[1])
    x = t[..., do[0] // 2 + 1:(-do[0]) // 2 + 1, do[1] // 2 + 1:(-do[1]) // 2 + 1,
          di[0] // 2 + 1:(-di[0]) // 2 + 1, di[1] // 2 + 1:(-di[1]) // 2 + 1]
    x = np.roll(x, (NO[0] // 2 + 1, NO[1] // 2 + 1, NI[0] // 2 + 1, NI[1] // 2 + 1),
                (-4, -3, -2, -1))
    return np.fft.fft2(np.fft.ifft2(x, axes=(-2, -1)), axes=(-4, -3)).real


def compute_mf(signal):
    """signal (C,2,24,24) -> pooling matrix (O=4096, K=400) float32."""
    spectrum = _signal_to_spectrum(signal.astype(np.float64))
    p0, p1 = spectrum.shape[-2], spectrum.shape[-1]
    a = _gw2d_algebra(spectrum)
    n = p0 * p1
    mat = a.reshape(a.shape[:-4] + (n, n))
    t = np.stack([scipy.linalg.expm(mat[i]) for i in range(mat.shape[0])])
    t = t.reshape(t.shape[:-2] + (p0, p1, p0, p1))
    m = _transform_to_map(t)
    return m.reshape(O, K).astype(np.float32)


# ------------------------------------------------------------ device kernel ---

_built = None


def _build():
    global _built
    if _built is not None:
        return _built
    nc = bacc.Bacc(dynamic_dma_scratch_size=256)
    f32 = mybir.dt.float32
    f32r = mybir.dt.float32r

    xT_d = nc.declare_dram_parameter("xT", (K, BS), f32r, isOutput=False)
    mfT_d = nc.declare_dram_parameter("mfT", (K, O), f32r, isOutput=False)
    out_d = nc.declare_dram_parameter("out", (BS, O), f32, isOutput=True)

    with tile.TileContext(nc) as tc:
        with tc.tile_pool(name="xpool", bufs=1) as xpool, \
             tc.tile_pool(name="wpool", bufs=1) as wpool, \
             tc.tile_pool(name="opool", bufs=3) as opool, \
             tc.tile_pool(name="ppool", bufs=8, space="PSUM") as ppool:
            # x shard first (small); mf arrives per-co-tile so matmuls start
            # after ~4MB instead of the full 8.2MB of loads.
            xt = xpool.tile([KP, KC, BS], f32r, name="xt")
            nc.sync.dma_start(xt[:], xT_d.rearrange("(c p) b -> p c b", p=KP))
            mts = []
            for co in range(O // OT):
                mt = wpool.tile([KP, KC, OT], f32r, tag=f"mt{co}", name=f"mt{co}")
                nc.sync.dma_start(
                    mt[:],
                    mfT_d[:, co * OT:(co + 1) * OT].rearrange(
                        "(c p) o -> p c o", p=KP))
                mts.append(mt)

            # G co-tiles share one output staging tile -> 1MB stores (fewer
            # SWDGE setups); copies alternate DVE/ACT so neither engine gates
            # PSUM-slot reuse.
            G = 4
            for cp in range(O // OT // G):
                for b in range(BS // BT):
                    ot = opool.tile([BT, G * OT], f32, name="ot")
                    for j in range(G):
                        co = cp * G + j
                        ps = ppool.tile([BT, OT], f32, name="ps")
                        for ci in range(KC):
                            nc.tensor.matmul(
                                ps[:],
                                xt[:, ci, b * BT:(b + 1) * BT],
                                mts[co][:, ci, :],
                                start=(ci == 0),
                                stop=(ci == KC - 1),
                            )
                        if j % 2 == 0:
                            nc.vector.tensor_copy(ot[:, j * OT:(j + 1) * OT], ps[:])
                        else:
                            nc.scalar.copy(ot[:, j * OT:(j + 1) * OT], ps[:])
                    nc.sync.dma_start(
                        out_d[b * BT:(b + 1) * BT, cp * G * OT:(cp + 1) * G * OT],
                        ot[:])
    nc.compile()
    _built = nc
    return nc


def _run(x, signal, **spmd_kwargs):
    nc = _build()
    mf = compute_mf(np.asarray(signal))
    mfT = np.ascontiguousarray(mf.T)                       # (400, 4096)
    xT = np.asarray(x).reshape(B, K).T                     # (400, 8192)
    in_maps = [
        {"xT": np.ascontiguousarray(xT[:, i * BS:(i + 1) * BS]), "mfT": mfT}
        for i in range(NCORES)
    ]
    res = run_bass_kernel_spmd(nc, in_maps, list(range(NCORES)), **spmd_kwargs)
    y = np.concatenate([r["out"] for r in res.results], axis=0)   # (8192, 4096)
    return y.reshape(B, C, NO[0], NO[1]), res


def kernel(x, signal):
    y, _ = _run(x, signal)
    return y



# revision 3
# speedup vs baseline: 1.9173x; 1.9173x over previous
"""GWPooling2D forward on 8 Trainium2 NeuronCores.

y[b, c, o] = sum_k m[c, o, k] * x[b, k]   (k = 400 input pixels, o = 256)

The pooling map m depends only on the small `signal` parameter and is
computed on host exactly as in the reference. It decomposes as

    m[c] = P0 + E[c]

where P0 (256 x 400) is the signal-independent resampling map (expm(0)=I
pushed through the same crop/roll/FFT pipeline) shared by all 16 channels,
and E[c] is the small per-channel correction (||E|| ~ 0.17 ||m||).

Device work per core (1024-batch shard, data parallel across 8 cores):
  yP = x_bf16 @ P0_bf16^T            (bf16 matmuls, 256 cols)
  yE = x_fp8  @ (E * s)_fp8^T        (fp8e4m3 DoubleRow matmuls, 4096 cols,
                                      K=400 in 2 packed chunks of 2x128/2x72)
yE is written back as fp8 (it is ~17% of y, so fp8 noise is ~0.6% of y),
yP as bf16; the host computes y = yP + yE/s. The single scale s keeps both
the quantized E and the yE PSUM values inside fp8e4m3 range (+-240).
"""

import numpy as np
import scipy.linalg

import concourse.bass as bass
import concourse.bacc as bacc
import concourse.mybir as mybir
import concourse.tile as tile
from concourse.bass_utils import run_bass_kernel_spmd
import ml_dtypes

C = 16
P = (24, 24)
NI = (20, 20)
NO = (16, 16)
B = 8192
NCORES = 8
BS = B // NCORES              # 1024 batch rows per core
K = NI[0] * NI[1]             # 400 contraction
O = NO[0] * NO[1]             # 256 output positions per channel
CO = C * O                    # 4096 (c,o) output columns
BT = 128                      # batch tile (PSUM partitions)
OT = 512                      # output-feature tile (PSUM free dim)
K0 = 256                      # DoubleRow chunk 0 (2 x 128 partitions)
K1 = K - K0                   # 144 = 2 x 72 partitions

F8 = ml_dtypes.float8_e4m3
BF16 = ml_dtypes.bfloat16


# ---------------------------------------------------------------- host map ---

def _hann(n):
    return 0.5 * (1.0 - np.cos(2.0 * np.pi * np.arange(n) / n))


def _signal_to_spectrum(signal):
    n0, n1 = signal.shape[-2], signal.shape[-1]
    window = _hann(n0)[:, None] * _hann(n1)[None, :]
    rx = np.arange((-n0) // 2 + 1, n0 // 2 + 1)[:, None]
    ry = np.arange((-n1) // 2 + 1, n1 // 2 + 1)[None, :]
    r = (1 + rx * rx + ry * ry).astype(np.float64)
    wf = np.roll(np.fft.fft2(signal), (n0 // 2, n1 // 2), (-2, -1)) / r / 5.0
    wt = np.fft.ifft2(np.roll(wf, (-(n0 // 2), -(n1 // 2)), (-2, -1))) * window
    return np.roll(np.fft.fft2(wt), (n0 // 2, n1 // 2), (-2, -1))


def _gw2d_algebra(w):
    p0, p1 = w.shape[-2], w.shape[-1]
    pad = [(0, 0)] * (w.ndim - 2) + [(p1 // 2, p1 // 2), (p0 // 2, p0 // 2)]
    wp = np.pad(w, pad)
    ia = np.arange(p0)[:, None] + np.arange(p0)[None, :]
    jb = np.arange(p1)[:, None] + np.arange(p1)[None, :]
    ws = wp[..., ia[:, None, :, None], jb[None, :, None, :]]
    ws = ws[..., ::-1, ::-1, :, :]
    kx = np.arange((-p0) // 2 + 1, p0 // 2 + 1)[:, None]
    ky = np.arange((-p1) // 2 + 1, p1 // 2 + 1)[None, :]
    return -1j * (ws[..., 0, :, :, :, :] * kx + ws[..., 1, :, :, :, :] * ky)


def _transform_to_map(t):
    p0, p1 = t.shape[-2], t.shape[-1]
    di = (p0 - NI[0], p1 - NI[1])
    do = (p0 - NO[0], p1 - NO[1])
    x = t[..., do[0] // 2 + 1:(-do[0]) // 2 + 1, do[1] // 2 + 1:(-do[1]) // 2 + 1,
          di[0] // 2 + 1:(-di[0]) // 2 + 1, di[1] // 2 + 1:(-di[1]) // 2 + 1]
    x = np.roll(x, (NO[0] // 2 + 1, NO[1] // 2 + 1, NI[0] // 2 + 1, NI[1] // 2 + 1),
                (-4, -3, -2, -1))
    return np.fft.fft2(np.fft.ifft2(x, axes=(-2, -1)), axes=(-4, -3)).real


def compute_mf(signal):
    """signal (C,2,24,24) -> pooling matrix (CO=4096, K=400) float32."""
    spectrum = _signal_to_spectrum(signal.astype(np.float64))
    p0, p1 = spectrum.shape[-2], spectrum.shape[-1]
    a = _gw2d_algebra(spectrum)
    n = p0 * p1
    mat = a.reshape(a.shape[:-4] + (n, n))
    t = np.stack([scipy.linalg.expm(mat[i]) for i in range(mat.shape[0])])
    t = t.reshape(t.shape[:-2] + (p0, p1, p0, p1))
    m = _transform_to_map(t)
    return m.reshape(CO, K).astype(np.float32)


_P0 = None


def compute_p0():
    """Signal-independent part of the map: expm(0)=I through the same
    crop/roll/FFT pipeline. (256, 400) float64."""
    global _P0
    if _P0 is None:
        t_id = np.eye(P[0] * P[1], dtype=np.complex128).reshape(
            1, P[0], P[1], P[0], P[1])
        _P0 = _transform_to_map(t_id).reshape(O, K)
    return _P0


# ------------------------------------------------------------ device kernel ---

_built = None


def _build():
    global _built
    if _built is not None:
        return _built
    nc = bacc.Bacc(dynamic_dma_scratch_size=256)
    f32 = mybir.dt.float32
    bf16 = mybir.dt.bfloat16
    f8 = mybir.dt.float8e4
    DR = mybir.MatmulPerfMode.DoubleRow

    xb_d = nc.declare_dram_parameter("xb", (K, BS), bf16, isOutput=False)
    p16_d = nc.declare_dram_parameter("p16", (K, O), bf16, isOutput=False)
    x80_d = nc.declare_dram_parameter("x80", (K0 // 2, 2, BS), f8, isOutput=False)
    x81_d = nc.declare_dram_parameter("x81", (K1 // 2, 2, BS), f8, isOutput=False)
    e80_d = nc.declare_dram_parameter("e80", (K0 // 2, 2, CO), f8, isOutput=False)
    e81_d = nc.declare_dram_parameter("e81", (K1 // 2, 2, CO), f8, isOutput=False)
    outE_d = nc.declare_dram_parameter("outE", (BS, CO), f8, isOutput=True)
    outP_d = nc.declare_dram_parameter("outP", (BS, O), bf16, isOutput=True)

    NB = BS // BT                 # 8 batch tiles
    NCO = CO // OT                # 8 E-column tiles
    G = 4                         # co-tiles per staged store
    KP = 100                      # bf16 path contraction chunk

    with tile.TileContext(nc) as tc:
        with tc.tile_pool(name="inpool", bufs=1) as inpool, \
             tc.tile_pool(name="opool", bufs=3) as opool, \
             tc.tile_pool(name="pppool", bufs=2, space="PSUM") as pppool, \
             tc.tile_pool(name="pepool", bufs=6, space="PSUM") as pepool:
            # ---- loads (program order == SP issue order) ----
            xb = inpool.tile([KP, K // KP, BS], bf16, name="xb")
            nc.sync.dma_start(xb[:], xb_d.rearrange("(c p) b -> p c b", p=KP))
            p16 = inpool.tile([KP, K // KP, O], bf16, name="p16")
            nc.sync.dma_start(p16[:], p16_d.rearrange("(c p) o -> p c o", p=KP))
            x80 = inpool.tile([K0 // 2, 2, BS], f8, name="x80")
            nc.sync.dma_start(x80[:], x80_d[:])
            x81 = inpool.tile([K1 // 2, 2, BS], f8, name="x81")
            nc.sync.dma_start(x81[:], x81_d[:])
            # E split in column halves so E matmuls start before all of E lands
            e80a = inpool.tile([K0 // 2, 2, CO // 2], f8, name="e80a")
            nc.sync.dma_start(e80a[:], e80_d[:, :, :CO // 2])
            e81a = inpool.tile([K1 // 2, 2, CO // 2], f8, name="e81a")
            nc.sync.dma_start(e81a[:], e81_d[:, :, :CO // 2])

            # ---- P part: yP = x_bf16 @ P0^T ----
            yps = opool.tile([BT, NB, O], bf16, tag="yps", name="yps")
            for b in range(NB):
                pp = pppool.tile([BT, O], f32, name="pp")
                for ci in range(K // KP):
                    nc.tensor.matmul(
                        pp[:],
                        xb[:, ci, b * BT:(b + 1) * BT],
                        p16[:, ci, :],
                        start=(ci == 0),
                        stop=(ci == K // KP - 1),
                    )
                if b % 2 == 0:
                    nc.scalar.copy(yps[:, b, :], pp[:])
                else:
                    nc.vector.tensor_copy(yps[:, b, :], pp[:])
            nc.sync.dma_start(outP_d.rearrange("(j p) o -> p j o", p=BT), yps[:])

            # second half of E while P part computes
            e80b = inpool.tile([K0 // 2, 2, CO // 2], f8, name="e80b")
            nc.sync.dma_start(e80b[:], e80_d[:, :, CO // 2:])
            e81b = inpool.tile([K1 // 2, 2, CO // 2], f8, name="e81b")
            nc.sync.dma_start(e81b[:], e81_d[:, :, CO // 2:])
            e80 = (e80a, e80b)
            e81 = (e81a, e81b)

            # ---- E part: yE = x_fp8 @ (E*s)^T, DoubleRow ----
            ncopy = 0
            for b in range(NB):
                for cp in range(NCO // G):
                    st = opool.tile([BT, G * OT], f8, name="st")
                    for j in range(G):
                        co = cp * G + j
                        half, cof = divmod(co * OT, CO // 2)
                        ps = pepool.tile([BT, OT], f32, name="ps")
                        nc.tensor.matmul(
                            ps[:],
                            x80[:, :, b * BT:(b + 1) * BT],
                            e80[half][:, :, cof:cof + OT],
                            start=True, stop=False, perf_mode=DR,
                        )
                        nc.tensor.matmul(
                            ps[:],
                            x81[:, :, b * BT:(b + 1) * BT],
                            e81[half][:, :, cof:cof + OT],
                            start=False, stop=True, perf_mode=DR,
                        )
                        if ncopy % 2 == 0:
                            nc.vector.tensor_copy(st[:, j * OT:(j + 1) * OT], ps[:])
                        else:
                            nc.scalar.copy(st[:, j * OT:(j + 1) * OT], ps[:])
                        ncopy += 1
                    nc.sync.dma_start(
                        outE_d[b * BT:(b + 1) * BT, cp * G * OT:(cp + 1) * G * OT],
                        st[:])
    nc.compile()
    _built = nc
    return nc


def _prep_host(x, signal):
    """Host-side factorization + quantization. Returns per-core input maps
    and the dequantization scale."""
    mf = compute_mf(np.asarray(signal))                     # (4096, 400)
    p0 = compute_p0()                                       # (256, 400) f64
    e = mf.astype(np.float64).reshape(C, O, K) - p0[None]
    ef = e.reshape(CO, K)

    # single scale: keeps E*s inside fp8 range and (with 8-sigma slack for
    # x ~ N(0,1)) the yE accumulator inside +-240 at the fp8 store
    row_norm = np.sqrt((ef * ef).sum(axis=1)).max()
    s = min(200.0 / np.abs(ef).max(), 200.0 / (8.0 * row_norm))
    e8 = (ef * s).astype(np.float32).astype(F8)             # (4096, 400)

    # DoubleRow packing: chunk0 k = i*128 + ki, chunk1 k = 256 + i*72 + ki
    e8c0 = np.ascontiguousarray(
        e8[:, :K0].reshape(CO, 2, K0 // 2).transpose(2, 1, 0))   # (128,2,4096)
    e8c1 = np.ascontiguousarray(
        e8[:, K0:].reshape(CO, 2, K1 // 2).transpose(2, 1, 0))   # (72,2,4096)

    p16 = np.ascontiguousarray(p0.T.astype(np.float32).astype(BF16))  # (400,256)

    xT = np.asarray(x).reshape(B, K).T                      # (400, 8192) f32
    xTb = xT.astype(BF16)
    x8 = xT.astype(F8)
    x8c0 = x8[:K0].reshape(2, K0 // 2, B).transpose(1, 0, 2)      # (128,2,8192)
    x8c1 = x8[K0:].reshape(2, K1 // 2, B).transpose(1, 0, 2)      # (72,2,8192)

    in_maps = []
    for i in range(NCORES):
        bs = slice(i * BS, (i + 1) * BS)
        in_maps.append({
            "xb": np.ascontiguousarray(xTb[:, bs]),
            "p16": p16,
            "x80": np.ascontiguousarray(x8c0[:, :, bs]),
            "x81": np.ascontiguousarray(x8c1[:, :, bs]),
            "e80": e8c0,
            "e81": e8c1,
        })
    return in_maps, s


def _run(x, signal, **spmd_kwargs):
    nc = _build()
    in_maps, s = _prep_host(x, signal)
    res = run_bass_kernel_spmd(nc, in_maps, list(range(NCORES)), **spmd_kwargs)
    parts = []
    for r in res.results:
        yE = r["outE"].astype(np.float32).reshape(BS, C, O) / s
        yP = r["outP"].astype(np.float32)
        parts.append(yE + yP[:, None, :])
    y = np.concatenate(parts, axis=0)
    return y.reshape(B, C, NO[0], NO[1]), res


def kernel(x, signal):
    y, _ = _run(x, signal)
    return y


# revision 5
# speedup vs baseline: 2.2315x; 1.1639x over previous
"""GWPooling2D forward on 8 Trainium2 NeuronCores.

y[b, c, o] = sum_k m[c, o, k] * x[b, k]   (k = 400 input pixels, o = 256)

The pooling map m depends only on the small `signal` parameter and is
computed on host exactly as in the reference. It decomposes as

    m[c] = P0 + E[c]

where P0 (256 x 400) is the signal-independent resampling map (expm(0)=I
pushed through the same crop/roll/FFT pipeline) shared by all 16 channels,
and E[c] is the small per-channel correction (||E|| ~ 0.17 ||m||).

Device work per core (1024-batch shard, data parallel across 8 cores):
  yP = x_bf16 @ P0_bf16^T            (bf16 matmuls, 256 cols)
  yE = x_fp8  @ (E * s)_fp8^T        (fp8e4m3 DoubleRow matmuls, 4096 cols,
                                      K=400 in 2 packed chunks of 2x128/2x72)
yE is written back as fp8 (it is ~17% of y, so fp8 noise is ~0.6% of y),
yP as bf16; the host computes y = yP + yE/s. The single scale s keeps both
the quantized E and the yE PSUM values inside fp8e4m3 range (+-240).
"""

import numpy as np
import scipy.linalg

import concourse.bass as bass
import concourse.bacc as bacc
import concourse.mybir as mybir
import concourse.tile as tile
from concourse.bass_utils import run_bass_kernel_spmd
import ml_dtypes

C = 16
P = (24, 24)
NI = (20, 20)
NO = (16, 16)
B = 8192
NCORES = 8
BS = B // NCORES              # 1024 batch rows per core
K = NI[0] * NI[1]             # 400 contraction
O = NO[0] * NO[1]             # 256 output positions per channel
CO = C * O                    # 4096 (c,o) output columns
BT = 128                      # batch tile (PSUM partitions)
OT = 512                      # output-feature tile (PSUM free dim)
K0 = 256                      # DoubleRow chunk 0 (2 x 128 partitions)
K1 = K - K0                   # 144 = 2 x 72 partitions

F8 = ml_dtypes.float8_e4m3
BF16 = ml_dtypes.bfloat16


# ---------------------------------------------------------------- host map ---

def _hann(n):
    return 0.5 * (1.0 - np.cos(2.0 * np.pi * np.arange(n) / n))


def _signal_to_spectrum(signal):
    n0, n1 = signal.shape[-2], signal.shape[-1]
    window = _hann(n0)[:, None] * _hann(n1)[None, :]
    rx = np.arange((-n0) // 2 + 1, n0 // 2 + 1)[:, None]
    ry = np.arange((-n1) // 2 + 1, n1 // 2 + 1)[None, :]
    r = (1 + rx * rx + ry * ry).astype(np.float64)
    wf = np.roll(np.fft.fft2(signal), (n0 // 2, n1 // 2), (-2, -1)) / r / 5.0
    wt = np.fft.ifft2(np.roll(wf, (-(n0 // 2), -(n1 // 2)), (-2, -1))) * window
    return np.roll(np.fft.fft2(wt), (n0 // 2, n1 // 2), (-2, -1))


def _gw2d_algebra(w):
    p0, p1 = w.shape[-2], w.shape[-1]
    pad = [(0, 0)] * (w.ndim - 2) + [(p1 // 2, p1 // 2), (p0 // 2, p0 // 2)]
    wp = np.pad(w, pad)
    ia = np.arange(p0)[:, None] + np.arange(p0)[None, :]
    jb = np.arange(p1)[:, None] + np.arange(p1)[None, :]
    ws = wp[..., ia[:, None, :, None], jb[None, :, None, :]]
    ws = ws[..., ::-1, ::-1, :, :]
    kx = np.arange((-p0) // 2 + 1, p0 // 2 + 1)[:, None]
    ky = np.arange((-p1) // 2 + 1, p1 // 2 + 1)[None, :]
    return -1j * (ws[..., 0, :, :, :, :] * kx + ws[..., 1, :, :, :, :] * ky)


def _transform_to_map(t):
    p0, p1 = t.shape[-2], t.shape[-1]
    di = (p0 - NI[0], p1 - NI[1])
    do = (p0 - NO[0], p1 - NO[1])
    x = t[..., do[0] // 2 + 1:(-do[0]) // 2 + 1, do[1] // 2 + 1:(-do[1]) // 2 + 1,
          di[0] // 2 + 1:(-di[0]) // 2 + 1, di[1] // 2 + 1:(-di[1]) // 2 + 1]
    x = np.roll(x, (NO[0] // 2 + 1, NO[1] // 2 + 1, NI[0] // 2 + 1, NI[1] // 2 + 1),
                (-4, -3, -2, -1))
    return np.fft.fft2(np.fft.ifft2(x, axes=(-2, -1)), axes=(-4, -3)).real


def compute_mf(signal):
    """signal (C,2,24,24) -> pooling matrix (CO=4096, K=400) float32."""
    spectrum = _signal_to_spectrum(signal.astype(np.float64))
    p0, p1 = spectrum.shape[-2], spectrum.shape[-1]
    a = _gw2d_algebra(spectrum)
    n = p0 * p1
    mat = a.reshape(a.shape[:-4] + (n, n))
    t = np.stack([scipy.linalg.expm(mat[i]) for i in range(mat.shape[0])])
    t = t.reshape(t.shape[:-2] + (p0, p1, p0, p1))
    m = _transform_to_map(t)
    return m.reshape(CO, K).astype(np.float32)


_P0 = None


def compute_p0():
    """Signal-independent part of the map: expm(0)=I through the same
    crop/roll/FFT pipeline. (256, 400) float64."""
    global _P0
    if _P0 is None:
        t_id = np.eye(P[0] * P[1], dtype=np.complex128).reshape(
            1, P[0], P[1], P[0], P[1])
        _P0 = _transform_to_map(t_id).reshape(O, K)
    return _P0


# ------------------------------------------------------------ device kernel ---

_built = None


def _build():
    global _built
    if _built is not None:
        return _built
    nc = bacc.Bacc(dynamic_dma_scratch_size=256)
    f32 = mybir.dt.float32
    bf16 = mybir.dt.bfloat16
    f8 = mybir.dt.float8e4
    DR = mybir.MatmulPerfMode.DoubleRow

    xb_d = nc.declare_dram_parameter("xb", (K, BS), bf16, isOutput=False)
    p16_d = nc.declare_dram_parameter("p16", (K, O), bf16, isOutput=False)
    x80_d = nc.declare_dram_parameter("x80", (K0 // 2, 2, BS), f8, isOutput=False)
    x81_d = nc.declare_dram_parameter("x81", (K1 // 2, 2, BS), f8, isOutput=False)
    e80_d = nc.declare_dram_parameter("e80", (K0 // 2, 2, CO), f8, isOutput=False)
    e81_d = nc.declare_dram_parameter("e81", (K1 // 2, 2, CO), f8, isOutput=False)
    outE_d = nc.declare_dram_parameter("outE", (BS, CO), f8, isOutput=True)
    outP_d = nc.declare_dram_parameter("outP", (BS, O), bf16, isOutput=True)

    NB = BS // BT                 # 8 batch tiles
    NCO = CO // OT                # 8 E-column tiles
    G = 4                         # co-tiles per staged store
    KP = 100                      # bf16 path contraction chunk

    with tile.TileContext(nc) as tc:
        with tc.tile_pool(name="inpool", bufs=1) as inpool, \
             tc.tile_pool(name="opool", bufs=4) as opool, \
             tc.tile_pool(name="pppool", bufs=2, space="PSUM") as pppool, \
             tc.tile_pool(name="pepool", bufs=3, space="PSUM") as pepool:
            # ---- E-path loads first so DoubleRow matmuls start ASAP ----
            x80 = inpool.tile([K0 // 2, 2, BS], f8, name="x80")
            nc.sync.dma_start(x80[:], x80_d[:])
            x81 = inpool.tile([K1 // 2, 2, BS], f8, name="x81")
            nc.sync.dma_start(x81[:], x81_d[:])
            # E is loaded in column quarters so matmuls start early
            NEQ = 4
            EQ = CO // NEQ                      # 1024 columns per quarter
            e80q, e81q = [], []
            for q in range(2):
                t0 = inpool.tile([K0 // 2, 2, EQ], f8, tag=f"e80q{q}",
                                 name=f"e80q{q}")
                nc.sync.dma_start(t0[:], e80_d[:, :, q * EQ:(q + 1) * EQ])
                e80q.append(t0)
                t1 = inpool.tile([K1 // 2, 2, EQ], f8, tag=f"e81q{q}",
                                 name=f"e81q{q}")
                nc.sync.dma_start(t1[:], e81_d[:, :, q * EQ:(q + 1) * EQ])
                e81q.append(t1)
            xb = inpool.tile([KP, K // KP, BS], bf16, name="xb")
            nc.sync.dma_start(xb[:], xb_d.rearrange("(c p) b -> p c b", p=KP))
            p16 = inpool.tile([KP, K // KP, O], bf16, name="p16")
            nc.sync.dma_start(p16[:], p16_d.rearrange("(c p) o -> p c o", p=KP))
            for q in range(2, NEQ):
                t0 = inpool.tile([K0 // 2, 2, EQ], f8, tag=f"e80q{q}",
                                 name=f"e80q{q}")
                nc.sync.dma_start(t0[:], e80_d[:, :, q * EQ:(q + 1) * EQ])
                e80q.append(t0)
                t1 = inpool.tile([K1 // 2, 2, EQ], f8, tag=f"e81q{q}",
                                 name=f"e81q{q}")
                nc.sync.dma_start(t1[:], e81_d[:, :, q * EQ:(q + 1) * EQ])
                e81q.append(t1)

            ncopy = 0

            def cast_copy(dst, src):
                nonlocal ncopy
                # ACT is a bit faster than DVE: give it 5 of every 9
                eng = (nc.vector.tensor_copy, nc.scalar.copy,
                       nc.scalar.copy, nc.vector.tensor_copy,
                       nc.scalar.copy, nc.vector.tensor_copy,
                       nc.scalar.copy, nc.vector.tensor_copy,
                       nc.scalar.copy)[ncopy % 9]
                eng(dst, src)
                ncopy += 1

            def e_group(b, cp):
                # one staging tile = 4 co-tiles = 2 double-bank PSUM tiles
                st = opool.tile([BT, G * OT], f8, name="st")
                for h in range(2):
                    ps = pepool.tile([BT, 2 * OT], f32, name="ps")
                    for j2 in range(2):
                        co = cp * G + h * 2 + j2
                        q, cof = divmod(co * OT, EQ)
                        nc.tensor.matmul(
                            ps[:, j2 * OT:(j2 + 1) * OT],
                            x80[:, :, b * BT:(b + 1) * BT],
                            e80q[q][:, :, cof:cof + OT],
                            start=True, stop=False, perf_mode=DR,
                        )
                        nc.tensor.matmul(
                            ps[:, j2 * OT:(j2 + 1) * OT],
                            x81[:, :, b * BT:(b + 1) * BT],
                            e81q[q][:, :, cof:cof + OT],
                            start=False, stop=True, perf_mode=DR,
                        )
                    cast_copy(st[:, h * 2 * OT:(h + 1) * 2 * OT], ps[:])
                nc.sync.dma_start(
                    outE_d[b * BT:(b + 1) * BT, cp * G * OT:(cp + 1) * G * OT],
                    st[:])

            # ---- E columns 0..2047 ----
            for b in range(NB):
                e_group(b, 0)

            # ---- P part: yP = x_bf16 @ P0^T ----
            yps = opool.tile([BT, NB, O], bf16, tag="yps", name="yps")
            for b in range(NB):
                pp = pppool.tile([BT, O], f32, name="pp")
                for ci in range(K // KP):
                    nc.tensor.matmul(
                        pp[:],
                        xb[:, ci, b * BT:(b + 1) * BT],
                        p16[:, ci, :],
                        start=(ci == 0),
                        stop=(ci == K // KP - 1),
                    )
                cast_copy(yps[:, b, :], pp[:])
            nc.sync.dma_start(outP_d.rearrange("(j p) o -> p j o", p=BT), yps[:])

            # ---- E columns 2048..4095 ----
            for b in range(NB):
                e_group(b, 1)
    nc.compile()
    _built = nc
    return nc


def _prep_host(x, signal):
    """Host-side factorization + quantization. Returns per-core input maps
    and the dequantization scale."""
    mf = compute_mf(np.asarray(signal))                     # (4096, 400)
    p0 = compute_p0()                                       # (256, 400) f64
    e = mf.astype(np.float64).reshape(C, O, K) - p0[None]
    ef = e.reshape(CO, K)

    # single scale: keeps E*s inside fp8 range and (with 8-sigma slack for
    # x ~ N(0,1)) the yE accumulator inside +-240 at the fp8 store
    row_norm = np.sqrt((ef * ef).sum(axis=1)).max()
    s = min(200.0 / np.abs(ef).max(), 200.0 / (8.0 * row_norm))
    e8 = (ef * s).astype(np.float32).astype(F8)             # (4096, 400)

    # DoubleRow packing: chunk0 k = i*128 + ki, chunk1 k = 256 + i*72 + ki
    e8c0 = np.ascontiguousarray(
        e8[:, :K0].reshape(CO, 2, K0 // 2).transpose(2, 1, 0))   # (128,2,4096)
    e8c1 = np.ascontiguousarray(
        e8[:, K0:].reshape(CO, 2, K1 // 2).transpose(2, 1, 0))   # (72,2,4096)

    p16 = np.ascontiguousarray(p0.T.astype(np.float32).astype(BF16))  # (400,256)

    xT = np.asarray(x).reshape(B, K).T                      # (400, 8192) f32
    xTb = xT.astype(BF16)
    x8 = xT.astype(F8)
    x8c0 = x8[:K0].reshape(2, K0 // 2, B).transpose(1, 0, 2)      # (128,2,8192)
    x8c1 = x8[K0:].reshape(2, K1 // 2, B).transpose(1, 0, 2)      # (72,2,8192)

    in_maps = []
    for i in range(NCORES):
        bs = slice(i * BS, (i + 1) * BS)
        in_maps.append({
            "xb": np.ascontiguousarray(xTb[:, bs]),
            "p16": p16,
            "x80": np.ascontiguousarray(x8c0[:, :, bs]),
            "x81": np.ascontiguousarray(x8c1[:, :, bs]),
            "e80": e8c0,
            "e81": e8c1,
        })
    return in_maps, s


def _run(x, signal, **spmd_kwargs):
    nc = _build()
    in_maps, s = _prep_host(x, signal)
    res = run_bass_kernel_spmd(nc, in_maps, list(range(NCORES)), **spmd_kwargs)
    parts = []
    for r in res.results:
        yE = r["outE"].astype(np.float32).reshape(BS, C, O) / s
        yP = r["outP"].astype(np.float32)
        parts.append(yE + yP[:, None, :])
    y = np.concatenate(parts, axis=0)
    return y.reshape(B, C, NO[0], NO[1]), res


def kernel(x, signal):
    y, _ = _run(x, signal)
    return y
